# revision 7
# baseline (speedup 1.0000x reference)
"""Trainium2 Bass kernel for nn_CrossAttention1D_78640851190158.

Math: k/v in the MHA come from a single cond token broadcast to all T
key positions, so the softmax over identical scores is exactly uniform
and the attention output equals v2 broadcast over T. The whole module
collapses to

    out[b, c, t] = x[b, c, t] + y[b, c]
    y[b] = W_eff @ cond[b] + b_eff

where W_eff = proj_w @ out_w @ wv2 @ Wv (wv2 = in_proj_w[2C:]) and
b_eff folds all the biases through the same chain. The LayerNorm / q
path contributes nothing to the output for ANY input values.

y (a [B, C] = [8, 512] matrix) is folded on the host together with the
weight chain; the device kernel is a pure memory-bound broadcast add
streaming x. I/O rides in bf16 (quantization rel-err ~1.6e-3, far
under the 2e-2 gate) to halve HBM traffic: 1 MiB in + 1 MiB out per
core.

Sharding: pure data parallelism over batch B=8 across the 8 cores.
Per core: x[b] viewed as [128, 4096] (partition p holds channels
4p..4p+3 as four 1024-wide quarters), y[b] as [128, 4].

Schedule: x loads on the sync HWDGE ring in CHUNKS pieces; the tiny y
load plus the stores ride the scalar HWDGE ring, so store descriptor
generation never queues behind loads and SDMA round-robins both rings
at packet granularity (loads and stores overlap).
"""

import os

import numpy as np

B, C, T, COND = 8, 512, 1024, 256
N_CORES = 8
P = 128
NQ = 4
QW = T                      # quarter width (cols per channel-quarter)
F = NQ * QW                 # 4096 cols per partition

# ---- tunables (A/B via env while iterating; defaults are the shipped config)
DTYPE = os.environ.get("K_DTYPE", "bf16")      # bf16 | f32
CHUNKS = int(os.environ.get("K_CHUNKS", "2"))  # x-load chunks (divides NQ)
SCHUNKS = int(os.environ.get("K_SCHUNKS", "2"))  # store chunks (divides NQ)

_cache = {}


def build_kernel(dtype=DTYPE, chunks=CHUNKS, schunks=SCHUNKS):
    import concourse.mybir as mybir
    from concourse import bacc

    dt = mybir.dt.bfloat16 if dtype == "bf16" else mybir.dt.float32
    nc = bacc.Bacc()

    f32 = mybir.dt.float32
    x_d = nc.dram_tensor("x", [P, F], dt, kind="ExternalInput")
    y_d = nc.dram_tensor("y", [P, NQ], f32, kind="ExternalInput")
    out_d = nc.dram_tensor("out", [P, F], dt, kind="ExternalOutput")

    cw = F // chunks            # cols per load chunk
    qpc = NQ // chunks          # quarters per load chunk
    sw = F // schunks           # cols per store chunk
    qps = NQ // schunks         # quarters per store chunk

    with (
        nc.Block() as block,
        nc.semaphore("s_y") as s_y,
        nc.semaphore("s_x0") as s_x0,
        nc.semaphore("s_x1") as s_x1,
        nc.semaphore("s_x2") as s_x2,
        nc.semaphore("s_x3") as s_x3,
        nc.semaphore("s_add") as s_add,
        nc.semaphore("s_out") as s_out,
        nc.sbuf_tensor("xt", [P, F], dt) as xt,
        nc.sbuf_tensor("y_sb", [P, NQ], f32) as y_sb,
    ):
        # Loads stay FIFO on the sync ring so early chunks complete early
        # (two queues round-robin at the SDMA engines, which delays the
        # first chunk — measured). Progressive sizes: small first chunk
        # starts the add/store pipeline early, small last chunk shortens
        # the drain tail. Stores go out per-quarter as each add retires,
        # alternating rings so descriptor-gen costs overlap.
        #   L0 = q0 (256 KiB), L1 = q1+q2 (512 KiB), L2 = q3 (256 KiB)
        #   S0, S2 -> scalar ring;  S1, S3 -> sync ring
        s_l = [s_x0, s_x1, s_x2]
        lchunks = [(0, 1), (1, 3), (3, 4)]  # quarter ranges per load chunk
        first_q = {lo: i for i, (lo, hi) in enumerate(lchunks)}

        @block.sync
        def _(sync):
            for i, (lo, hi) in enumerate(lchunks):
                sync.dma_start(
                    out=xt[:, lo * QW : hi * QW],
                    in_=x_d[:, lo * QW : hi * QW],
                ).then_inc(s_l[i], 16)
            for q in (1, 3):
                sync.wait_ge(s_add, q + 1)
                sync.dma_start(
                    out=out_d[:, q * QW : (q + 1) * QW],
                    in_=xt[:, q * QW : (q + 1) * QW],
                ).then_inc(s_out, 16)
            sync.wait_ge(s_out, 64)

        @block.vector
        def _(vector):
            vector.wait_ge(s_y, 16)
            for q in range(NQ):
                if q in first_q:
                    vector.wait_ge(s_l[first_q[q]], 16)
                vector.tensor_scalar_add(
                    out=xt[:, q * QW : (q + 1) * QW],
                    in0=xt[:, q * QW : (q + 1) * QW],
                    scalar1=y_sb[:, q : q + 1],
                ).then_inc(s_add, 1)

        @block.scalar
        def _(scalar):
            scalar.dma_start(out=y_sb[:], in_=y_d[:]).then_inc(s_y, 16)
            for q in (0, 2):
                scalar.wait_ge(s_add, q + 1)
                scalar.dma_start(
                    out=out_d[:, q * QW : (q + 1) * QW],
                    in_=xt[:, q * QW : (q + 1) * QW],
                ).then_inc(s_out, 16)
            scalar.wait_ge(s_out, 64)

    nc.compile()
    return nc


def fold_weights(Wv, bv, in_proj_w, in_proj_b, out_w, out_b, proj_w, proj_b):
    """Fold the v-path weight chain into one [C, COND] map (float64)."""
    wv2 = np.asarray(in_proj_w, np.float64)[2 * C :]
    bv2 = np.asarray(in_proj_b, np.float64)[2 * C :]
    Wv = np.asarray(Wv, np.float64)
    bv = np.asarray(bv, np.float64)
    out_w = np.asarray(out_w, np.float64)
    out_b = np.asarray(out_b, np.float64)
    proj_w = np.asarray(proj_w, np.float64)
    proj_b = np.asarray(proj_b, np.float64)

    po = proj_w @ out_w
    W_eff = po @ wv2 @ Wv
    b_eff = proj_b + proj_w @ out_b + po @ bv2 + po @ wv2 @ bv
    return W_eff, b_eff


def prepare_in_maps(inputs, dtype=DTYPE):
    import ml_dtypes

    np_dt = ml_dtypes.bfloat16 if dtype == "bf16" else np.float32
    x = np.asarray(inputs["x"], np.float32)
    cond = np.asarray(inputs["cond"], np.float64)
    W_eff, b_eff = fold_weights(
        inputs["Wv"], inputs["bv"], inputs["in_proj_w"], inputs["in_proj_b"],
        inputs["out_w"], inputs["out_b"], inputs["proj_w"], inputs["proj_b"],
    )
    y = (cond @ W_eff.T + b_eff).astype(np.float32)     # [B, C]
    in_maps = []
    for b in range(B):
        in_maps.append({
            "x": np.ascontiguousarray(x[b].reshape(P, F).astype(np_dt)),
            "y": np.ascontiguousarray(y[b].reshape(P, NQ)),
        })
    return in_maps


def kernel(**inputs):
    from concourse.bass_utils import run_bass_kernel_spmd

    if "nc" not in _cache:
        _cache["nc"] = build_kernel()
    nc = _cache["nc"]
    in_maps = prepare_in_maps(inputs)
    res = run_bass_kernel_spmd(nc, in_maps, list(range(N_CORES)))
    out = np.stack(
        [np.asarray(r["out"], np.float32).reshape(C, T) for r in res.results]
    )
    return out.astype(np.float32)


# revision 10
# speedup vs baseline: 1.0947x; 1.0947x over previous
"""Trainium2 Bass kernel for nn_CrossAttention1D_78640851190158.

Math: k/v in the MHA come from a single cond token broadcast to all T
key positions, so the softmax over identical scores is exactly uniform
and the attention output equals v2 broadcast over T. The whole module
collapses to

    out[b, c, t] = x[b, c, t] + y[b, c]
    y[b] = W_eff @ cond[b] + b_eff

where W_eff = proj_w @ out_w @ wv2 @ Wv (wv2 = in_proj_w[2C:]) and
b_eff folds all the biases through the same chain. The LayerNorm / q
path contributes nothing to the output for ANY input values.

y (a [B, C] = [8, 512] matrix) is folded on the host together with the
weight chain; the device kernel is a pure memory-bound broadcast add
streaming x. I/O rides in bf16 (quantization rel-err ~1.6e-3, far
under the 2e-2 gate) to halve HBM traffic: 1 MiB in + 1 MiB out per
core.

Sharding: pure data parallelism over batch B=8 across the 8 cores.
Per core: x[b] viewed as [128, 4096] (partition p holds channels
4p..4p+3 as four 1024-wide quarters), y[b] as [128, 4].

Schedule: x loads on the sync HWDGE ring in CHUNKS pieces; the tiny y
load plus the stores ride the scalar HWDGE ring, so store descriptor
generation never queues behind loads and SDMA round-robins both rings
at packet granularity (loads and stores overlap).
"""

import os

import numpy as np

B, C, T, COND = 8, 512, 1024, 256
N_CORES = 8
P = 128
NQ = 4
QW = T                      # quarter width (cols per channel-quarter)
F = NQ * QW                 # 4096 cols per partition

# ---- tunables (A/B via env while iterating; defaults are the shipped config)
DTYPE = os.environ.get("K_DTYPE", "bf16")      # bf16 | f32
CHUNKS = int(os.environ.get("K_CHUNKS", "2"))  # x-load chunks (divides NQ)
SCHUNKS = int(os.environ.get("K_SCHUNKS", "2"))  # store chunks (divides NQ)

_cache = {}


def build_kernel(dtype=DTYPE, chunks=CHUNKS, schunks=SCHUNKS):
    import concourse.mybir as mybir
    from concourse import bacc

    dt = mybir.dt.bfloat16 if dtype == "bf16" else mybir.dt.float32
    nc = bacc.Bacc()

    f32 = mybir.dt.float32
    x_d = nc.dram_tensor("x", [P, F], dt, kind="ExternalInput")
    y_d = nc.dram_tensor("y", [P, NQ], f32, kind="ExternalInput")
    out_d = nc.dram_tensor("out", [P, F], dt, kind="ExternalOutput")

    cw = F // chunks            # cols per load chunk
    qpc = NQ // chunks          # quarters per load chunk
    sw = F // schunks           # cols per store chunk
    qps = NQ // schunks         # quarters per store chunk

    with (
        nc.Block() as block,
        nc.semaphore("s_y") as s_y,
        nc.semaphore("s_x0") as s_x0,
        nc.semaphore("s_x1") as s_x1,
        nc.semaphore("s_x2") as s_x2,
        nc.semaphore("s_x3") as s_x3,
        nc.semaphore("s_add") as s_add,
        nc.semaphore("s_out") as s_out,
        nc.sbuf_tensor("xt", [P, F], dt) as xt,
        nc.sbuf_tensor("y_sb", [P, NQ], f32) as y_sb,
    ):
        # Loads ride the SWDGE (gpsimd/Q7) queue — a third descriptor
        # channel that measured ~341 GB/s for 4 KB+ descriptors, vs the
        # ~215 GB/s cap of one HWDGE ring. FIFO on one queue keeps chunk 0
        # completing first. Store half 0 goes out early on the scalar HWDGE
        # ring (overlaps the chunk-1 load); store half 1 rides the SWDGE
        # queue right behind the loads.
        s_l = [s_x0, s_x1]
        CW = 2 * QW  # 2048 cols per chunk

        @block.sync
        def _(sync):
            sync.dma_start(
                out=xt[:, 0:CW], in_=x_d[:, 0:CW]
            ).then_inc(s_l[0], 16)
            sync.wait_ge(s_add, 4)
            sync.dma_start(
                out=out_d[:, CW : 2 * CW],
                in_=xt[:, CW : 2 * CW],
            ).then_inc(s_out, 16)
            sync.wait_ge(s_out, 32)

        @block.gpsimd
        def _(gpsimd):
            gpsimd.dma_start(
                out=xt[:, CW : 2 * CW],
                in_=x_d[:, CW : 2 * CW],
            ).then_inc(s_l[1], 16)

        @block.vector
        def _(vector):
            vector.wait_ge(s_y, 16)
            for q in range(NQ):
                if q % 2 == 0:
                    vector.wait_ge(s_l[q // 2], 16)
                vector.tensor_scalar_add(
                    out=xt[:, q * QW : (q + 1) * QW],
                    in0=xt[:, q * QW : (q + 1) * QW],
                    scalar1=y_sb[:, q : q + 1],
                ).then_inc(s_add, 1)

        @block.scalar
        def _(scalar):
            scalar.dma_start(out=y_sb[:], in_=y_d[:]).then_inc(s_y, 16)
            scalar.wait_ge(s_add, 2)
            scalar.dma_start(
                out=out_d[:, 0:CW],
                in_=xt[:, 0:CW],
            ).then_inc(s_out, 16)
            scalar.wait_ge(s_out, 32)

    nc.compile()
    return nc


def fold_weights(Wv, bv, in_proj_w, in_proj_b, out_w, out_b, proj_w, proj_b):
    """Fold the v-path weight chain into one [C, COND] map (float64)."""
    wv2 = np.asarray(in_proj_w, np.float64)[2 * C :]
    bv2 = np.asarray(in_proj_b, np.float64)[2 * C :]
    Wv = np.asarray(Wv, np.float64)
    bv = np.asarray(bv, np.float64)
    out_w = np.asarray(out_w, np.float64)
    out_b = np.asarray(out_b, np.float64)
    proj_w = np.asarray(proj_w, np.float64)
    proj_b = np.asarray(proj_b, np.float64)

    po = proj_w @ out_w
    W_eff = po @ wv2 @ Wv
    b_eff = proj_b + proj_w @ out_b + po @ bv2 + po @ wv2 @ bv
    return W_eff, b_eff


def prepare_in_maps(inputs, dtype=DTYPE):
    import ml_dtypes

    np_dt = ml_dtypes.bfloat16 if dtype == "bf16" else np.float32
    x = np.asarray(inputs["x"], np.float32)
    cond = np.asarray(inputs["cond"], np.float64)
    W_eff, b_eff = fold_weights(
        inputs["Wv"], inputs["bv"], inputs["in_proj_w"], inputs["in_proj_b"],
        inputs["out_w"], inputs["out_b"], inputs["proj_w"], inputs["proj_b"],
    )
    y = (cond @ W_eff.T + b_eff).astype(np.float32)     # [B, C]
    in_maps = []
    for b in range(B):
        in_maps.append({
            "x": np.ascontiguousarray(x[b].reshape(P, F).astype(np_dt)),
            "y": np.ascontiguousarray(y[b].reshape(P, NQ)),
        })
    return in_maps


def kernel(**inputs):
    from concourse.bass_utils import run_bass_kernel_spmd

    if "nc" not in _cache:
        _cache["nc"] = build_kernel()
    nc = _cache["nc"]
    in_maps = prepare_in_maps(inputs)
    res = run_bass_kernel_spmd(nc, in_maps, list(range(N_CORES)))
    out = np.stack(
        [np.asarray(r["out"], np.float32).reshape(C, T) for r in res.results]
    )
    return out.astype(np.float32)
